# revision 21
# baseline (speedup 1.0000x reference)
"""Trainium2 Bass kernel for nn_Coefficients: assemble the sparse circuit
coefficient matrix

    out = [ kcl  = [ M | 0 ]                       (N rows)
            kvl  = [ 0 | I_E | -M^T ]              (E rows)
            elem = diag(z) / diag(y) scatter ]     (E rows)

Row-wise shard of M across 8 NeuronCores: core d loads its 256-row shard
M[d*256:(d+1)*256, :] from HBM ONCE and derives both output blocks from it:
  - kcl:  the shard itself, cast to fp16 (SBUF->DRAM)
  - mt:   -shard^T via PE transpose = the 256-COLUMN slice
          [4096, 256] of -M^T (column-sharded kvl right block)
  - bands: eye / diag(z) / diag(y) from params/kinds, fused in one store.
This cuts per-core HBM traffic from 16 MiB (baseline: shard read twice +
two f32 writes) to ~8.3 MiB (one f32 read + fp16 writes), the binding
constraint at the ~358 GB/s per-core HBM limit.  fp16 carries 11
significand bits -> max rel err ~4.9e-4 on the value-carrying blocks,
well inside the 2e-2 gate; the host widens fp16->f32 during placement
(an exact cast).

Layout notes: small params/kinds loads go at the HWDGE ring heads (the
rings are FIFO; queued behind the megabyte loads they arrive ~15us late
and stall the DVE band chain — and SWDGE is worse); no gpsimd DMAs at
all.  Instruction/semaphore count is kept low (the teardown sem-sweep
is serial per engine): 4 big loads, 4 kcl stores, 4 mt stores, 1 band
store, 8 fused [128,1024] PSUM->SBUF copies on 8 distinct PSUM banks
(bufs=8 -> the PE never stalls waiting for a bank to drain).
The host unshards by pure indexing (mt arrives as [q, (c j)] and is
un-interleaved with reshape/transpose; all numeric content is
device-produced).
"""

import numpy as np

N = 2048
E = 4096
W = 2 * E + N  # 10240
D = 8
NR = N // D  # 256 kcl rows / mt columns per core
EC = E // D  # 512 band elems per core

_CACHE: dict = {}


def _build(opts=None):
    import concourse.bacc as bacc
    import concourse.tile as tile
    import concourse.mybir as mybir
    from concourse._compat import get_trn_type

    opts = dict(opts or {})
    ppool_bufs = opts.get("ppool_bufs", 8)

    f32 = mybir.dt.float32
    f16 = mybir.dt.float16
    i32 = mybir.dt.int32

    nc = bacc.Bacc(
        get_trn_type() or "TRN2",
        target_bir_lowering=False,
        debug=False,
        enable_asserts=False,
        num_devices=D,
    )

    m_rows = nc.dram_tensor("m_rows", [NR, E], f32, kind="ExternalInput")
    params_s = nc.dram_tensor("params_s", [128, 4], f32, kind="ExternalInput")
    kinds_s = nc.dram_tensor("kinds_s", [128, 4], i32, kind="ExternalInput")

    kcl = nc.dram_tensor("kcl", [NR, E], f16, kind="ExternalOutput")
    # mt layout [q, (c j)]: mt[q, c*256+j] = -M[d*256+j, c*128+q]; host
    # reshape(128,32,256).transpose(1,0,2).reshape(4096,256) -> -M^T cols
    mt = nc.dram_tensor("mt", [128, 32 * NR], f16, kind="ExternalOutput")
    # bands [128, 1152]: [0:128] eye, [128:640] diag(z), [640:1152] diag(y),
    # each as 4 side-by-side [128,128] chunks (elem index = c*128 + p)
    bands = nc.dram_tensor("bands", [128, 1152], f16, kind="ExternalOutput")

    AO = mybir.AluOpType
    ACT_COPY = mybir.ActivationFunctionType.Copy
    H = E // 2

    with tile.TileContext(nc) as tc:
        with (
            tc.tile_pool(name="cpool", bufs=1) as cpool,
            tc.tile_pool(name="ppool", bufs=ppool_bufs, space="PSUM") as ppool,
        ):
            # ---- tiny inputs at the ring heads (each HWDGE ring is FIFO:
            # queued behind the 4 MiB loads they would arrive ~15us late
            # and stall the band chain)
            pt = cpool.tile([128, 4], f32)
            kti = cpool.tile([128, 4], i32)
            nc.sync.dma_start(out=pt[:], in_=params_s.ap()[:, :])
            nc.scalar.dma_start(out=kti[:], in_=kinds_s.ap()[:, :])

            # ---- shard loads, 1 MiB chunks on both HWDGE rings
            in0 = cpool.tile([128, E], f32, tag="in0")  # shard rows 0..127
            in1 = cpool.tile([128, E], f32, tag="in1")  # shard rows 128..255
            nc.sync.dma_start(out=in0[:, 0:H], in_=m_rows.ap()[0:128, 0:H])
            nc.scalar.dma_start(out=in1[:, 0:H], in_=m_rows.ap()[128:256, 0:H])
            nc.sync.dma_start(out=in0[:, H:E], in_=m_rows.ap()[0:128, H:E])
            nc.scalar.dma_start(out=in1[:, H:E], in_=m_rows.ap()[128:256, H:E])

            # ---- fp16 identity (PE transpose operand + eye-band payload)
            ident = cpool.tile([128, 128], f16)
            nc.gpsimd.memset(ident[:], 0.0)
            nc.gpsimd.affine_select(
                out=ident[:],
                in_=ident[:],
                compare_op=AO.not_equal,
                fill=1.0,
                base=0,
                pattern=[[-1, 128]],
                channel_multiplier=1,
            )

            # ---- z/y diagonal values (layout r = c*128 + p)
            ktf = cpool.tile([128, 4], f32)
            rm = cpool.tile([128, 4], f32)
            im = cpool.tile([128, 4], f32)
            vm = cpool.tile([128, 4], f32)
            sm = cpool.tile([128, 4], f32)
            onm = cpool.tile([128, 4], f32)
            offm = cpool.tile([128, 4], f32)
            zv = cpool.tile([128, 4], f32)
            yv = cpool.tile([128, 4], f32)
            t0 = cpool.tile([128, 4], f32)
            t1 = cpool.tile([128, 4], f32)
            nc.vector.tensor_copy(ktf[:], kti[:])
            nc.vector.tensor_scalar(rm[:], ktf[:], 0.0, None, op0=AO.is_equal)
            nc.vector.tensor_scalar(im[:], ktf[:], 1.0, None, op0=AO.is_equal)
            nc.vector.tensor_scalar(vm[:], ktf[:], 2.0, None, op0=AO.is_equal)
            nc.vector.tensor_scalar(sm[:], ktf[:], 3.0, None, op0=AO.is_equal)
            nc.vector.tensor_scalar(onm[:], pt[:], 0.0, None, op0=AO.is_gt)
            nc.vector.tensor_scalar(offm[:], pt[:], 0.0, None, op0=AO.is_le)
            # z = vc + sw*off - r*params
            nc.vector.tensor_tensor(t0[:], sm[:], offm[:], op=AO.mult)
            nc.vector.tensor_tensor(t0[:], vm[:], t0[:], op=AO.add)
            nc.vector.tensor_tensor(t1[:], rm[:], pt[:], op=AO.mult)
            nc.vector.tensor_tensor(zv[:], t0[:], t1[:], op=AO.subtract)
            # y = r + ivs + sw*on
            nc.vector.tensor_tensor(t0[:], sm[:], onm[:], op=AO.mult)
            nc.vector.tensor_tensor(t0[:], im[:], t0[:], op=AO.add)
            nc.vector.tensor_tensor(yv[:], rm[:], t0[:], op=AO.add)

            # ---- band assembly: [eye | diag(z) chunks | diag(y) chunks]
            # (all on gpsimd, which is otherwise idle -> DVE stays free for
            # the cast/copy critical path)
            bt = cpool.tile([128, 1152], f16)
            nc.vector.tensor_copy(bt[:, 0:128], ident[:])
            for c in range(4):
                nc.vector.tensor_scalar(
                    bt[:, 128 + c * 128 : 256 + c * 128], ident[:],
                    zv[:, c : c + 1], None, op0=AO.mult,
                )
                nc.vector.tensor_scalar(
                    bt[:, 640 + c * 128 : 768 + c * 128], ident[:],
                    yv[:, c : c + 1], None, op0=AO.mult,
                )
            nc.sync.dma_start(out=bands.ap()[:, :], in_=bt[:])

            # ---- shard cast f32 -> fp16 (DVE), chunk-matched to the loads
            h0 = cpool.tile([128, E], f16, tag="h0")
            h1 = cpool.tile([128, E], f16, tag="h1")
            nc.vector.tensor_copy(h0[:, 0:H], in0[:, 0:H])
            nc.vector.tensor_copy(h1[:, 0:H], in1[:, 0:H])
            nc.vector.tensor_copy(h0[:, H:E], in0[:, H:E])
            nc.vector.tensor_copy(h1[:, H:E], in1[:, H:E])

            # ---- kcl stores from the cast shard (0.5 MiB chunks so store
            # work is ring-ready early, keeping HBM busy once loads drain)
            nc.sync.dma_start(out=kcl.ap()[0:128, 0:H], in_=h0[:, 0:H])
            nc.scalar.dma_start(out=kcl.ap()[128:256, 0:H], in_=h1[:, 0:H])
            nc.sync.dma_start(out=kcl.ap()[0:128, H:E], in_=h0[:, H:E])
            nc.scalar.dma_start(out=kcl.ap()[128:256, H:E], in_=h1[:, H:E])

            # ---- -M^T: 32 col-chunks x 2 row-halves of PE transpose into
            # [128,1024] PSUM banks (4 chunks each); negate folded into the
            # fused PSUM->SBUF copies (DVE/ACT alternate)
            stg = [
                cpool.tile([128, 16 * NR], f16, name=f"stg{t}", tag=f"stg{t}")
                for t in range(2)
            ]
            for q in range(8):
                ps = ppool.tile([128, 1024], f16)
                for k in range(4):
                    c = 4 * q + k
                    nc.tensor.transpose(
                        out=ps[:, k * 256 : k * 256 + 128],
                        in_=h0[:, c * 128 : (c + 1) * 128],
                        identity=ident[:],
                    )
                    nc.tensor.transpose(
                        out=ps[:, k * 256 + 128 : (k + 1) * 256],
                        in_=h1[:, c * 128 : (c + 1) * 128],
                        identity=ident[:],
                    )
                dst = stg[q // 4][:, (q % 4) * 1024 : (q % 4 + 1) * 1024]
                # alternate DVE/ACT so consecutive copies run on different
                # engines in parallel (8 PSUM banks -> the PE never waits
                # on a copy to reuse a bank)
                if q % 2 == 0:
                    nc.vector.tensor_scalar(dst, ps[:], -1.0, None, op0=AO.mult)
                else:
                    nc.scalar.activation(dst, ps[:], ACT_COPY, scale=-1.0)
                if q % 4 == 3:
                    t = q // 4
                    eng = nc.sync if t == 0 else nc.scalar
                    eng.dma_start(
                        out=mt.ap()[:, t * 4096 : (t + 1) * 4096], in_=stg[t][:]
                    )


    nc.compile()
    return nc


def _get_nc(opts=None):
    key = ("nc", tuple(sorted((opts or {}).items())))
    if key not in _CACHE:
        _CACHE[key] = _build(opts)
    return _CACHE[key]


def _in_maps(M, params, kinds):
    maps = []
    for d in range(D):
        maps.append(
            {
                "m_rows": np.ascontiguousarray(M[d * NR : (d + 1) * NR, :]),
                "params_s": np.ascontiguousarray(
                    params[d * EC : (d + 1) * EC].reshape(4, 128).T
                ),
                "kinds_s": np.ascontiguousarray(
                    kinds[d * EC : (d + 1) * EC].reshape(4, 128).T
                ),
            }
        )
    return maps


def kernel(M, params, kinds, _trace=False, _trace_kwargs=None, _opts=None):
    from concourse.bass_utils import run_bass_kernel_spmd

    M = np.ascontiguousarray(np.asarray(M, dtype=np.float32))
    params = np.ascontiguousarray(np.asarray(params, dtype=np.float32))
    kinds = np.ascontiguousarray(np.asarray(kinds, dtype=np.int32))
    assert M.shape == (N, E) and params.shape == (E,) and kinds.shape == (E,)

    nc = _get_nc(_opts)
    res = run_bass_kernel_spmd(
        nc,
        _in_maps(M, params, kinds),
        core_ids=list(range(D)),
        trace=_trace,
        **(_trace_kwargs or {}),
    )
    out = np.zeros((N + 2 * E, W), np.float32)
    for d in range(D):
        r = res.results[d]
        out[d * NR : (d + 1) * NR, 0:E] = r["kcl"]
        out[N : N + E, 2 * E + d * NR : 2 * E + (d + 1) * NR] = (
            r["mt"].reshape(128, 32, NR).transpose(1, 0, 2).reshape(E, NR)
        )
        b = r["bands"]
        eye3 = b[:, 0:128]
        zb3 = b[:, 128:640].reshape(128, 4, 128)
        yb3 = b[:, 640:1152].reshape(128, 4, 128)
        for c in range(4):
            g0 = d * EC + c * 128  # global elem index of band start
            out[N + g0 : N + g0 + 128, E + g0 : E + g0 + 128] = eye3
            out[N + E + g0 : N + E + g0 + 128, g0 : g0 + 128] = zb3[:, c, :]
            out[N + E + g0 : N + E + g0 + 128, E + g0 : E + g0 + 128] = yb3[:, c, :]
    if _trace:
        _CACHE["last_result"] = res
    return out


# revision 22
# speedup vs baseline: 1.0883x; 1.0883x over previous
"""Trainium2 Bass kernel for nn_Coefficients: assemble the sparse circuit
coefficient matrix

    out = [ kcl  = [ M | 0 ]                       (N rows)
            kvl  = [ 0 | I_E | -M^T ]              (E rows)
            elem = diag(z) / diag(y) scatter ]     (E rows)

Row-wise shard of M across 8 NeuronCores: core d loads its 256-row shard
M[d*256:(d+1)*256, :] from HBM ONCE and derives both output blocks from it:
  - kcl:  the shard itself, cast to fp16 (SBUF->DRAM)
  - mt:   -shard^T via PE transpose = the 256-COLUMN slice
          [4096, 256] of -M^T (column-sharded kvl right block)
  - bands: eye / diag(z) / diag(y) from params/kinds, fused in one store.
This cuts per-core HBM traffic from 16 MiB (baseline: shard read twice +
two f32 writes) to ~8.3 MiB (one f32 read + fp16 writes), the binding
constraint at the ~358 GB/s per-core HBM limit.  fp16 carries 11
significand bits -> max rel err ~4.9e-4 on the value-carrying blocks,
well inside the 2e-2 gate; the host widens fp16->f32 during placement
(an exact cast).

Layout notes: small params/kinds loads go at the HWDGE ring heads (the
rings are FIFO; queued behind the megabyte loads they arrive ~15us late
and stall the DVE band chain — and SWDGE is worse); no gpsimd DMAs at
all.  Instruction/semaphore count is kept low (the teardown sem-sweep
is serial per engine): 4 big loads, 4 kcl stores, 4 mt stores, 1 band
store, 8 fused [128,1024] PSUM->SBUF copies on 8 distinct PSUM banks
(bufs=8 -> the PE never stalls waiting for a bank to drain).
The host unshards by pure indexing (mt arrives as [q, (c j)] and is
un-interleaved with reshape/transpose; all numeric content is
device-produced).
"""

import numpy as np

N = 2048
E = 4096
W = 2 * E + N  # 10240
D = 8
NR = N // D  # 256 kcl rows / mt columns per core
EC = E // D  # 512 band elems per core

_CACHE: dict = {}


def _build(opts=None):
    import concourse.bacc as bacc
    import concourse.tile as tile
    import concourse.mybir as mybir
    from concourse._compat import get_trn_type

    opts = dict(opts or {})
    ppool_bufs = opts.get("ppool_bufs", 8)

    f32 = mybir.dt.float32
    f16 = mybir.dt.float16
    i32 = mybir.dt.int32

    nc = bacc.Bacc(
        get_trn_type() or "TRN2",
        target_bir_lowering=False,
        debug=False,
        enable_asserts=False,
        num_devices=D,
    )

    m_rows = nc.dram_tensor("m_rows", [NR, E], f32, kind="ExternalInput")
    params_s = nc.dram_tensor("params_s", [128, 4], f32, kind="ExternalInput")
    kinds_s = nc.dram_tensor("kinds_s", [128, 4], i32, kind="ExternalInput")

    kcl = nc.dram_tensor("kcl", [NR, E], f16, kind="ExternalOutput")
    # mt layout [q, (c j)]: mt[q, c*256+j] = -M[d*256+j, c*128+q]; host
    # reshape(128,32,256).transpose(1,0,2).reshape(4096,256) -> -M^T cols
    mt = nc.dram_tensor("mt", [128, 32 * NR], f16, kind="ExternalOutput")
    # bands [128, 1152]: [0:128] eye, [128:640] diag(z), [640:1152] diag(y),
    # each as 4 side-by-side [128,128] chunks (elem index = c*128 + p)
    bands = nc.dram_tensor("bands", [128, 1152], f16, kind="ExternalOutput")

    AO = mybir.AluOpType
    ACT_COPY = mybir.ActivationFunctionType.Copy
    H = E // 2

    with tile.TileContext(nc) as tc:
        with (
            tc.tile_pool(name="cpool", bufs=1) as cpool,
            tc.tile_pool(name="ppool", bufs=ppool_bufs, space="PSUM") as ppool,
        ):
            # ---- tiny inputs at the ring heads (each HWDGE ring is FIFO:
            # queued behind the 4 MiB loads they would arrive ~15us late
            # and stall the band chain)
            pt = cpool.tile([128, 4], f32)
            kti = cpool.tile([128, 4], i32)
            nc.sync.dma_start(out=pt[:], in_=params_s.ap()[:, :])
            nc.sync.dma_start(out=kti[:], in_=kinds_s.ap()[:, :])

            # ---- shard loads, 1 MiB chunks on both HWDGE rings
            in0 = cpool.tile([128, E], f32, tag="in0")  # shard rows 0..127
            in1 = cpool.tile([128, E], f32, tag="in1")  # shard rows 128..255
            nc.sync.dma_start(out=in0[:, 0:H], in_=m_rows.ap()[0:128, 0:H])
            nc.scalar.dma_start(out=in1[:, 0:H], in_=m_rows.ap()[128:256, 0:H])
            nc.sync.dma_start(out=in0[:, H:E], in_=m_rows.ap()[0:128, H:E])
            nc.scalar.dma_start(out=in1[:, H:E], in_=m_rows.ap()[128:256, H:E])

            # ---- fp16 identity (PE transpose operand + eye-band payload)
            ident = cpool.tile([128, 128], f16)
            nc.gpsimd.memset(ident[:], 0.0)
            nc.gpsimd.affine_select(
                out=ident[:],
                in_=ident[:],
                compare_op=AO.not_equal,
                fill=1.0,
                base=0,
                pattern=[[-1, 128]],
                channel_multiplier=1,
            )

            # ---- z/y diagonal values (layout r = c*128 + p)
            ktf = cpool.tile([128, 4], f32)
            rm = cpool.tile([128, 4], f32)
            im = cpool.tile([128, 4], f32)
            vm = cpool.tile([128, 4], f32)
            sm = cpool.tile([128, 4], f32)
            onm = cpool.tile([128, 4], f32)
            offm = cpool.tile([128, 4], f32)
            zv = cpool.tile([128, 4], f32)
            yv = cpool.tile([128, 4], f32)
            t0 = cpool.tile([128, 4], f32)
            t1 = cpool.tile([128, 4], f32)
            nc.vector.tensor_copy(ktf[:], kti[:])
            nc.vector.tensor_scalar(rm[:], ktf[:], 0.0, None, op0=AO.is_equal)
            nc.vector.tensor_scalar(im[:], ktf[:], 1.0, None, op0=AO.is_equal)
            nc.vector.tensor_scalar(vm[:], ktf[:], 2.0, None, op0=AO.is_equal)
            nc.vector.tensor_scalar(sm[:], ktf[:], 3.0, None, op0=AO.is_equal)
            nc.vector.tensor_scalar(onm[:], pt[:], 0.0, None, op0=AO.is_gt)
            nc.vector.tensor_scalar(offm[:], pt[:], 0.0, None, op0=AO.is_le)
            # z = vc + sw*off - r*params
            nc.vector.tensor_tensor(t0[:], sm[:], offm[:], op=AO.mult)
            nc.vector.tensor_tensor(t0[:], vm[:], t0[:], op=AO.add)
            nc.vector.tensor_tensor(t1[:], rm[:], pt[:], op=AO.mult)
            nc.vector.tensor_tensor(zv[:], t0[:], t1[:], op=AO.subtract)
            # y = r + ivs + sw*on
            nc.vector.tensor_tensor(t0[:], sm[:], onm[:], op=AO.mult)
            nc.vector.tensor_tensor(t0[:], im[:], t0[:], op=AO.add)
            nc.vector.tensor_tensor(yv[:], rm[:], t0[:], op=AO.add)

            # ---- band assembly: [eye | diag(z) chunks | diag(y) chunks]
            # (all on gpsimd, which is otherwise idle -> DVE stays free for
            # the cast/copy critical path)
            bt = cpool.tile([128, 1152], f16)
            nc.vector.tensor_copy(bt[:, 0:128], ident[:])
            for c in range(4):
                nc.vector.tensor_scalar(
                    bt[:, 128 + c * 128 : 256 + c * 128], ident[:],
                    zv[:, c : c + 1], None, op0=AO.mult,
                )
                nc.vector.tensor_scalar(
                    bt[:, 640 + c * 128 : 768 + c * 128], ident[:],
                    yv[:, c : c + 1], None, op0=AO.mult,
                )
            nc.sync.dma_start(out=bands.ap()[:, :], in_=bt[:])

            # ---- shard cast f32 -> fp16 (DVE), chunk-matched to the loads
            h0 = cpool.tile([128, E], f16, tag="h0")
            h1 = cpool.tile([128, E], f16, tag="h1")
            nc.vector.tensor_copy(h0[:, 0:H], in0[:, 0:H])
            nc.vector.tensor_copy(h1[:, 0:H], in1[:, 0:H])
            nc.vector.tensor_copy(h0[:, H:E], in0[:, H:E])
            nc.vector.tensor_copy(h1[:, H:E], in1[:, H:E])

            # ---- kcl stores from the cast shard (0.5 MiB chunks so store
            # work is ring-ready early, keeping HBM busy once loads drain)
            nc.sync.dma_start(out=kcl.ap()[0:128, 0:H], in_=h0[:, 0:H])
            nc.scalar.dma_start(out=kcl.ap()[128:256, 0:H], in_=h1[:, 0:H])
            nc.sync.dma_start(out=kcl.ap()[0:128, H:E], in_=h0[:, H:E])
            nc.scalar.dma_start(out=kcl.ap()[128:256, H:E], in_=h1[:, H:E])

            # ---- -M^T: 32 col-chunks x 2 row-halves of PE transpose into
            # [128,1024] PSUM banks (4 chunks each); negate folded into the
            # fused PSUM->SBUF copies (DVE/ACT alternate)
            stg = [
                cpool.tile([128, 16 * NR], f16, name=f"stg{t}", tag=f"stg{t}")
                for t in range(2)
            ]
            for q in range(8):
                ps = ppool.tile([128, 1024], f16)
                for k in range(4):
                    c = 4 * q + k
                    nc.tensor.transpose(
                        out=ps[:, k * 256 : k * 256 + 128],
                        in_=h0[:, c * 128 : (c + 1) * 128],
                        identity=ident[:],
                    )
                    nc.tensor.transpose(
                        out=ps[:, k * 256 + 128 : (k + 1) * 256],
                        in_=h1[:, c * 128 : (c + 1) * 128],
                        identity=ident[:],
                    )
                dst = stg[q // 4][:, (q % 4) * 1024 : (q % 4 + 1) * 1024]
                # alternate DVE/ACT so consecutive copies run on different
                # engines in parallel (8 PSUM banks -> the PE never waits
                # on a copy to reuse a bank)
                if q % 2 == 0:
                    nc.vector.tensor_scalar(dst, ps[:], -1.0, None, op0=AO.mult)
                else:
                    nc.scalar.activation(dst, ps[:], ACT_COPY, scale=-1.0)
                if q % 4 == 3:
                    t = q // 4
                    eng = nc.sync if t == 0 else nc.scalar
                    eng.dma_start(
                        out=mt.ap()[:, t * 4096 : (t + 1) * 4096], in_=stg[t][:]
                    )


    nc.compile()
    return nc


def _get_nc(opts=None):
    key = ("nc", tuple(sorted((opts or {}).items())))
    if key not in _CACHE:
        _CACHE[key] = _build(opts)
    return _CACHE[key]


def _in_maps(M, params, kinds):
    maps = []
    for d in range(D):
        maps.append(
            {
                "m_rows": np.ascontiguousarray(M[d * NR : (d + 1) * NR, :]),
                "params_s": np.ascontiguousarray(
                    params[d * EC : (d + 1) * EC].reshape(4, 128).T
                ),
                "kinds_s": np.ascontiguousarray(
                    kinds[d * EC : (d + 1) * EC].reshape(4, 128).T
                ),
            }
        )
    return maps


def kernel(M, params, kinds, _trace=False, _trace_kwargs=None, _opts=None):
    from concourse.bass_utils import run_bass_kernel_spmd

    M = np.ascontiguousarray(np.asarray(M, dtype=np.float32))
    params = np.ascontiguousarray(np.asarray(params, dtype=np.float32))
    kinds = np.ascontiguousarray(np.asarray(kinds, dtype=np.int32))
    assert M.shape == (N, E) and params.shape == (E,) and kinds.shape == (E,)

    nc = _get_nc(_opts)
    res = run_bass_kernel_spmd(
        nc,
        _in_maps(M, params, kinds),
        core_ids=list(range(D)),
        trace=_trace,
        **(_trace_kwargs or {}),
    )
    out = np.zeros((N + 2 * E, W), np.float32)
    for d in range(D):
        r = res.results[d]
        out[d * NR : (d + 1) * NR, 0:E] = r["kcl"]
        out[N : N + E, 2 * E + d * NR : 2 * E + (d + 1) * NR] = (
            r["mt"].reshape(128, 32, NR).transpose(1, 0, 2).reshape(E, NR)
        )
        b = r["bands"]
        eye3 = b[:, 0:128]
        zb3 = b[:, 128:640].reshape(128, 4, 128)
        yb3 = b[:, 640:1152].reshape(128, 4, 128)
        for c in range(4):
            g0 = d * EC + c * 128  # global elem index of band start
            out[N + g0 : N + g0 + 128, E + g0 : E + g0 + 128] = eye3
            out[N + E + g0 : N + E + g0 + 128, g0 : g0 + 128] = zb3[:, c, :]
            out[N + E + g0 : N + E + g0 + 128, E + g0 : E + g0 + 128] = yb3[:, c, :]
    if _trace:
        _CACHE["last_result"] = res
    return out


# revision 29
# speedup vs baseline: 1.1370x; 1.0448x over previous
"""Trainium2 Bass kernel for nn_Coefficients: assemble the sparse circuit
coefficient matrix

    out = [ kcl  = [ M | 0 ]                       (N rows)
            kvl  = [ 0 | I_E | -M^T ]              (E rows)
            elem = diag(z) / diag(y) scatter ]     (E rows)

Row-wise shard of M across 8 NeuronCores: core d loads its 256-row shard
M[d*256:(d+1)*256, :] from HBM ONCE and derives both output blocks from it:
  - kcl:  the shard itself, cast to fp16 (SBUF->DRAM)
  - mt:   -shard^T via PE transpose = the 256-COLUMN slice
          [4096, 256] of -M^T (column-sharded kvl right block)
  - bands: eye / diag(z) / diag(y) from params/kinds, fused in one store.
This cuts per-core HBM traffic from 16 MiB (baseline: shard read twice +
two f32 writes) to ~8.3 MiB (one f32 read + fp16 writes), the binding
constraint at the ~358 GB/s per-core HBM limit.  fp16 carries 11
significand bits -> max rel err ~4.9e-4 on the value-carrying blocks,
well inside the 2e-2 gate; the host widens fp16->f32 during placement
(an exact cast).

Layout notes: the params/kinds shard rides one packed [128,8] f32 DMA
at the sync ring head (the rings are FIFO; queued behind the megabyte
loads it would arrive ~15us late and stall the DVE band chain — and
SWDGE is worse); no gpsimd DMAs at all.  Instruction/semaphore count is kept low (the teardown sem-sweep
is serial per engine): 4 big loads, 4 kcl stores, 2 mt stores, 1 band
store (issued mid-stream so its completion receipt is hidden), 8 fused
[128,1024] PSUM->SBUF copies on 8 distinct PSUM banks (bufs=8 -> the
PE never stalls waiting for a bank to drain).
The host unshards by pure indexing (mt arrives as [q, (c j)] and is
un-interleaved with reshape/transpose; all numeric content is
device-produced).
"""

import numpy as np

N = 2048
E = 4096
W = 2 * E + N  # 10240
D = 8
NR = N // D  # 256 kcl rows / mt columns per core
EC = E // D  # 512 band elems per core

_CACHE: dict = {}


def _build(opts=None):
    import concourse.bacc as bacc
    import concourse.tile as tile
    import concourse.mybir as mybir
    from concourse._compat import get_trn_type

    opts = dict(opts or {})
    ppool_bufs = opts.get("ppool_bufs", 8)

    f32 = mybir.dt.float32
    f16 = mybir.dt.float16
    i32 = mybir.dt.int32

    nc = bacc.Bacc(
        get_trn_type() or "TRN2",
        target_bir_lowering=False,
        debug=False,
        enable_asserts=False,
        num_devices=D,
    )

    m_rows = nc.dram_tensor("m_rows", [NR, E], f32, kind="ExternalInput")
    # params and kinds packed side by side: [:, 0:4] params f32,
    # [:, 4:8] kinds exactly cast to f32 (values 0..3) -> one tiny DMA
    pk_s = nc.dram_tensor("pk_s", [128, 8], f32, kind="ExternalInput")

    kcl = nc.dram_tensor("kcl", [NR, E], f16, kind="ExternalOutput")
    # mt layout [q, (c j)]: mt[q, c*256+j] = -M[d*256+j, c*128+q]; host
    # reshape(128,32,256).transpose(1,0,2).reshape(4096,256) -> -M^T cols
    mt = nc.dram_tensor("mt", [128, 32 * NR], f16, kind="ExternalOutput")
    # bands [128, 1152]: [0:128] eye, [128:640] diag(z), [640:1152] diag(y),
    # each as 4 side-by-side [128,128] chunks (elem index = c*128 + p)
    bands = nc.dram_tensor("bands", [128, 1152], f16, kind="ExternalOutput")

    AO = mybir.AluOpType
    ACT_COPY = mybir.ActivationFunctionType.Copy
    H = E // 2

    with tile.TileContext(nc) as tc:
        with (
            tc.tile_pool(name="cpool", bufs=1) as cpool,
            tc.tile_pool(name="ppool", bufs=ppool_bufs, space="PSUM") as ppool,
        ):
            # ---- tiny packed input at the sync ring head (the HWDGE rings
            # are FIFO: queued behind the 4 MiB loads it would arrive ~15us
            # late and stall the band chain)
            pk = cpool.tile([128, 8], f32)
            nc.sync.dma_start(out=pk[:], in_=pk_s.ap()[:, :])

            # ---- shard loads, 1 MiB chunks on both HWDGE rings
            in0 = cpool.tile([128, E], f32, tag="in0")  # shard rows 0..127
            in1 = cpool.tile([128, E], f32, tag="in1")  # shard rows 128..255
            nc.sync.dma_start(out=in0[:, 0:H], in_=m_rows.ap()[0:128, 0:H])
            nc.scalar.dma_start(out=in1[:, 0:H], in_=m_rows.ap()[128:256, 0:H])
            nc.sync.dma_start(out=in0[:, H:E], in_=m_rows.ap()[0:128, H:E])
            nc.scalar.dma_start(out=in1[:, H:E], in_=m_rows.ap()[128:256, H:E])

            # ---- fp16 identity (PE transpose operand + eye-band payload)
            ident = cpool.tile([128, 128], f16)
            nc.gpsimd.memset(ident[:], 0.0)
            nc.gpsimd.affine_select(
                out=ident[:],
                in_=ident[:],
                compare_op=AO.not_equal,
                fill=1.0,
                base=0,
                pattern=[[-1, 128]],
                channel_multiplier=1,
            )

            # ---- z/y diagonal values (layout r = c*128 + p)
            rm = cpool.tile([128, 4], f32)
            im = cpool.tile([128, 4], f32)
            vm = cpool.tile([128, 4], f32)
            sm = cpool.tile([128, 4], f32)
            onm = cpool.tile([128, 4], f32)
            offm = cpool.tile([128, 4], f32)
            zv = cpool.tile([128, 4], f32)
            yv = cpool.tile([128, 4], f32)
            t0 = cpool.tile([128, 4], f32)
            t1 = cpool.tile([128, 4], f32)
            ktf = pk[:, 4:8]
            pt = pk[:, 0:4]
            nc.vector.tensor_scalar(rm[:], ktf, 0.0, None, op0=AO.is_equal)
            nc.vector.tensor_scalar(im[:], ktf, 1.0, None, op0=AO.is_equal)
            nc.vector.tensor_scalar(vm[:], ktf, 2.0, None, op0=AO.is_equal)
            nc.vector.tensor_scalar(sm[:], ktf, 3.0, None, op0=AO.is_equal)
            nc.vector.tensor_scalar(onm[:], pt, 0.0, None, op0=AO.is_gt)
            nc.vector.tensor_scalar(offm[:], pt, 0.0, None, op0=AO.is_le)
            # z = vc + sw*off - r*params
            nc.vector.tensor_tensor(t0[:], sm[:], offm[:], op=AO.mult)
            nc.vector.tensor_tensor(t0[:], vm[:], t0[:], op=AO.add)
            nc.vector.tensor_tensor(t1[:], rm[:], pt, op=AO.mult)
            nc.vector.tensor_tensor(zv[:], t0[:], t1[:], op=AO.subtract)
            # y = r + ivs + sw*on
            nc.vector.tensor_tensor(t0[:], sm[:], onm[:], op=AO.mult)
            nc.vector.tensor_tensor(t0[:], im[:], t0[:], op=AO.add)
            nc.vector.tensor_tensor(yv[:], rm[:], t0[:], op=AO.add)

            # ---- band assembly: [eye | diag(z) chunks | diag(y) chunks]
            # (on DVE before the casts: it fills the window while the big
            # loads are still in flight; gpsimd's per-op overhead is ~5x)
            bt = cpool.tile([128, 1152], f16)
            nc.vector.tensor_copy(bt[:, 0:128], ident[:])
            for c in range(4):
                nc.vector.tensor_scalar(
                    bt[:, 128 + c * 128 : 256 + c * 128], ident[:],
                    zv[:, c : c + 1], None, op0=AO.mult,
                )
                nc.vector.tensor_scalar(
                    bt[:, 640 + c * 128 : 768 + c * 128], ident[:],
                    yv[:, c : c + 1], None, op0=AO.mult,
                )
            nc.sync.dma_start(out=bands.ap()[:, :], in_=bt[:])

            # ---- shard cast f32 -> fp16 (DVE), chunk-matched to the loads
            h0 = cpool.tile([128, E], f16, tag="h0")
            h1 = cpool.tile([128, E], f16, tag="h1")
            nc.vector.tensor_copy(h0[:, 0:H], in0[:, 0:H])
            nc.vector.tensor_copy(h1[:, 0:H], in1[:, 0:H])
            nc.vector.tensor_copy(h0[:, H:E], in0[:, H:E])
            nc.vector.tensor_copy(h1[:, H:E], in1[:, H:E])

            # ---- kcl stores from the cast shard (0.5 MiB chunks so store
            # work is ring-ready early, keeping HBM busy once loads drain)
            nc.sync.dma_start(out=kcl.ap()[0:128, 0:H], in_=h0[:, 0:H])
            nc.scalar.dma_start(out=kcl.ap()[128:256, 0:H], in_=h1[:, 0:H])
            nc.sync.dma_start(out=kcl.ap()[0:128, H:E], in_=h0[:, H:E])
            nc.scalar.dma_start(out=kcl.ap()[128:256, H:E], in_=h1[:, H:E])

            # ---- -M^T: 32 col-chunks x 2 row-halves of PE transpose into
            # [128,1024] PSUM banks (4 chunks each); negate folded into the
            # fused PSUM->SBUF copies (DVE/ACT alternate)
            stg = [
                cpool.tile([128, 16 * NR], f16, name=f"stg{t}", tag=f"stg{t}")
                for t in range(2)
            ]
            for q in range(8):
                ps = ppool.tile([128, 1024], f16)
                for k in range(4):
                    c = 4 * q + k
                    nc.tensor.transpose(
                        out=ps[:, k * 256 : k * 256 + 128],
                        in_=h0[:, c * 128 : (c + 1) * 128],
                        identity=ident[:],
                    )
                    nc.tensor.transpose(
                        out=ps[:, k * 256 + 128 : (k + 1) * 256],
                        in_=h1[:, c * 128 : (c + 1) * 128],
                        identity=ident[:],
                    )
                dst = stg[q // 4][:, (q % 4) * 1024 : (q % 4 + 1) * 1024]
                # alternate DVE/ACT so consecutive copies run on different
                # engines in parallel (8 PSUM banks -> the PE never waits
                # on a copy to reuse a bank)
                if q % 2 == 0:
                    nc.vector.tensor_scalar(dst, ps[:], -1.0, None, op0=AO.mult)
                else:
                    nc.scalar.activation(dst, ps[:], ACT_COPY, scale=-1.0)
                if q % 4 == 3:
                    t = q // 4
                    eng = nc.sync if t == 0 else nc.scalar
                    eng.dma_start(
                        out=mt.ap()[:, t * 4096 : (t + 1) * 4096], in_=stg[t][:]
                    )


    nc.compile()
    return nc


def _get_nc(opts=None):
    key = ("nc", tuple(sorted((opts or {}).items())))
    if key not in _CACHE:
        _CACHE[key] = _build(opts)
    return _CACHE[key]


def _in_maps(M, params, kinds):
    maps = []
    for d in range(D):
        maps.append(
            {
                "m_rows": np.ascontiguousarray(M[d * NR : (d + 1) * NR, :]),
                "pk_s": np.ascontiguousarray(
                    np.concatenate(
                        [
                            params[d * EC : (d + 1) * EC].reshape(4, 128).T,
                            kinds[d * EC : (d + 1) * EC]
                            .reshape(4, 128)
                            .T.astype(np.float32),
                        ],
                        axis=1,
                    )
                ),
            }
        )
    return maps


def kernel(M, params, kinds, _trace=False, _trace_kwargs=None, _opts=None):
    from concourse.bass_utils import run_bass_kernel_spmd

    M = np.ascontiguousarray(np.asarray(M, dtype=np.float32))
    params = np.ascontiguousarray(np.asarray(params, dtype=np.float32))
    kinds = np.ascontiguousarray(np.asarray(kinds, dtype=np.int32))
    assert M.shape == (N, E) and params.shape == (E,) and kinds.shape == (E,)

    nc = _get_nc(_opts)
    res = run_bass_kernel_spmd(
        nc,
        _in_maps(M, params, kinds),
        core_ids=list(range(D)),
        trace=_trace,
        **(_trace_kwargs or {}),
    )
    out = np.zeros((N + 2 * E, W), np.float32)
    for d in range(D):
        r = res.results[d]
        out[d * NR : (d + 1) * NR, 0:E] = r["kcl"]
        out[N : N + E, 2 * E + d * NR : 2 * E + (d + 1) * NR] = (
            r["mt"].reshape(128, 32, NR).transpose(1, 0, 2).reshape(E, NR)
        )
        b = r["bands"]
        eye3 = b[:, 0:128]
        zb3 = b[:, 128:640].reshape(128, 4, 128)
        yb3 = b[:, 640:1152].reshape(128, 4, 128)
        for c in range(4):
            g0 = d * EC + c * 128  # global elem index of band start
            out[N + g0 : N + g0 + 128, E + g0 : E + g0 + 128] = eye3
            out[N + E + g0 : N + E + g0 + 128, g0 : g0 + 128] = zb3[:, c, :]
            out[N + E + g0 : N + E + g0 + 128, E + g0 : E + g0 + 128] = yb3[:, c, :]
    if _trace:
        _CACHE["last_result"] = res
    return out
